# revision 4
# baseline (speedup 1.0000x reference)
"""Trainium2 Bass kernel for nn_CausalSelfAttention_74268574482879.

The reference module's attention scores are overwritten by the causal mask
(q/k are discarded), so softmax weights are uniform over positions <= t:
    y = cummean_T(x) @ W_v @ W_p,   W_v = w_attn[:, 1024:1536]

Distribution: the 4096 rows of (B*T) are split into 8 chunks of 512 rows,
one per NeuronCore.  The only cross-chunk dependency is the column-sum of
all preceding rows in the same batch element; the host passes that tiny
(512,) halo vector per core while slicing the shards.

Per-core dataflow (matmuls keep operands in natural layout — the PE's
implicit transpose of the stationary operand does all layout work):
  stage A: lhsT=x_tile, rhs=U_scaled  ->  psA = scale*(local cumsum)^T (PSUM)
           tile colsums are recovered from psA's last column (one fused
           tensor_scalar each), then a DVE/GpSimd carry adds P_j[c]*scale[t]
  stage B: lhsT=Wv,     rhs=A^T       ->  M1^T = (A @ Wv)^T
  stage C: lhsT=M1^T,   rhs=Wp        ->  Y = M1 @ Wp   (natural, DMA out)
"""

import numpy as np

import concourse.bass as bass
import concourse.bacc as bacc
import concourse.mybir as mybir
import concourse.tile as tile
from concourse import bass_utils

N_CORES = 8
B, T, C = 2, 2048, 512
CHUNK = 512               # rows of flattened (B*T) per core
P = 128
NT = CHUNK // P           # 4 row-tiles per chunk
NI = C // P               # 4 col-tiles of the 512 feature dim
F32 = mybir.dt.float32
F32R = mybir.dt.float32r
BF16 = mybir.dt.bfloat16

MODE = ["f32r"]           # "f32" | "f32r" | "bf16" (stage B/C dtype)
TRACE = [False]
LAST_RESULT = [None]
_STATE = {}


def _build_nc(mode):
    nc = bacc.Bacc(
        "TRN2", target_bir_lowering=False, debug=False, num_devices=N_CORES
    )
    # dtype plan per mode: a_dt feeds stage-A matmuls, bc_dt feeds B/C.
    # float32r keeps fp32 bits but runs the PE in single-pass reduced mode;
    # the verifier wants every producer of a matmul operand to declare it.
    if mode == "f32":
        a_dt, bc_dt = F32, F32
    elif mode == "f32r":
        a_dt, bc_dt = F32R, F32R
    else:  # bf16 B/C, f32r stage A
        a_dt, bc_dt = F32R, BF16
    bc_bf16 = bc_dt == BF16
    wdma_dt = F32 if bc_bf16 else bc_dt

    x_d = nc.dram_tensor("x", (CHUNK, C), a_dt, kind="ExternalInput")
    wv_d = nc.dram_tensor("wv", (C, C), wdma_dt, kind="ExternalInput")
    wp_d = nc.dram_tensor("wp", (C, C), wdma_dt, kind="ExternalInput")
    us_d = nc.dram_tensor("us", (P, CHUNK), a_dt, kind="ExternalInput")
    sc_d = nc.dram_tensor("sc", (P, CHUNK), F32, kind="ExternalInput")
    pc_d = nc.dram_tensor("pc", (P, NI), F32, kind="ExternalInput")
    rc_d = nc.dram_tensor("rc", (P, NT), F32, kind="ExternalInput")
    y_d = nc.dram_tensor("y", (CHUNK, C), F32, kind="ExternalOutput")

    x_ap, wv_ap, wp_ap = x_d.ap(), wv_d.ap(), wp_d.ap()
    us_ap, sc_ap, pc_ap, rc_ap, y_ap = (
        us_d.ap(), sc_d.ap(), pc_d.ap(), rc_d.ap(), y_d.ap()
    )

    with tile.TileContext(nc) as tc:
        with (
            tc.tile_pool(name="io", bufs=1) as io,
            tc.tile_pool(name="tmp", bufs=4) as tmp_pool,
            tc.tile_pool(name="psbig", bufs=8, space="PSUM") as ps_pool,
        ):
            # ---- inputs to SBUF (order = DMA priority) ----
            x_sb = []
            for k in range(NT):
                xk = io.tile([P, C], a_dt, name=f"x{k}")
                nc.sync.dma_start(xk[:], x_ap[k * P : (k + 1) * P, :])
                x_sb.append(xk)
            us_sb = io.tile([P, CHUNK], a_dt, name="us_sb")
            nc.sync.dma_start(us_sb[:], us_ap[:, :])
            pc_sb = io.tile([P, NI], F32, name="pc_sb")
            nc.sync.dma_start(pc_sb[:], pc_ap[:, :])
            rc_sb = io.tile([P, NT], F32, name="rc_sb")
            nc.sync.dma_start(rc_sb[:], rc_ap[:, :])
            sc_sb = io.tile([P, CHUNK], F32, name="sc_sb")
            nc.sync.dma_start(sc_sb[:], sc_ap[:, :])
            wv_sb, wp_sb = [], []
            for i in range(NI):
                wvi = io.tile([P, C], wdma_dt, name=f"wv{i}")
                nc.sync.dma_start(wvi[:], wv_ap[i * P : (i + 1) * P, :])
                wv_sb.append(wvi)
            for j in range(NI):
                wpj = io.tile([P, C], wdma_dt, name=f"wp{j}")
                nc.sync.dma_start(wpj[:], wp_ap[j * P : (j + 1) * P, :])
                wp_sb.append(wpj)

            if bc_bf16:
                wvb, wpb = [], []
                for i in range(NI):
                    t = io.tile([P, C], BF16, name=f"wvb{i}")
                    nc.scalar.copy(t[:], wv_sb[i][:])
                    wvb.append(t)
                for j in range(NI):
                    t = io.tile([P, C], BF16, name=f"wpb{j}")
                    nc.scalar.copy(t[:], wp_sb[j][:])
                    wpb.append(t)
            else:
                wvb, wpb = wv_sb, wp_sb

            # ---- stage A: scaled local cumsum (PE), j-outer so the first
            # round only needs x_sb[0] ----
            psA = [
                ps_pool.tile([P, CHUNK], F32, name=f"psA{i}", tag="big")
                for i in range(NI)
            ]
            for j in range(NT):
                tj = slice(j * P, (j + 1) * P)
                for i in range(NI):
                    ci = slice(i * P, (i + 1) * P)
                    nc.tensor.matmul(
                        psA[i][:, tj],
                        x_sb[j][:, ci],
                        us_sb[:, tj],
                        start=True,
                        stop=True,
                    )

            # ---- running prefixes Pc[:, i*NT+j] = p + sum_{k<j} colsum_k,
            # recovering colsum_k from psA's last column of tile k ----
            Pc_sb = io.tile([P, NT * NI], F32, name="Pc_sb")
            for i in range(NI):
                nc.vector.tensor_copy(
                    Pc_sb[:, i * NT : i * NT + 1], pc_sb[:, i : i + 1]
                )
                for j in range(1, NT):
                    lcol = (j - 1) * P + P - 1
                    nc.vector.tensor_scalar(
                        Pc_sb[:, i * NT + j : i * NT + j + 1],
                        psA[i][:, lcol : lcol + 1],
                        rc_sb[:, j - 1 : j],
                        Pc_sb[:, i * NT + j - 1 : i * NT + j],
                        mybir.AluOpType.mult,
                        mybir.AluOpType.add,
                    )

            # ---- carry: A^T = psA + Pc * scale  (GpSimd mul + DVE add) ----
            A_sb = []
            for i in range(NI):
                ai = io.tile([P, CHUNK], bc_dt, name=f"A{i}")
                for j in range(NT):
                    tj = slice(j * P, (j + 1) * P)
                    col = i * NT + j
                    tmp = tmp_pool.tile([P, P], F32, name="carry", tag="carry")
                    nc.gpsimd.tensor_scalar_mul(
                        tmp[:], sc_sb[:, tj], Pc_sb[:, col : col + 1]
                    )
                    nc.vector.tensor_add(ai[:, tj], psA[i][:, tj], tmp[:])
                A_sb.append(ai)

            # ---- stage B: M1^T = (A @ Wv)^T ----
            M1_sb = []
            for jj in range(NI):
                psm = ps_pool.tile([P, CHUNK], F32, name=f"psM{jj}", tag="big")
                cj = slice(jj * P, (jj + 1) * P)
                for i in range(NI):
                    nc.tensor.matmul(
                        psm[:],
                        wvb[i][:, cj],
                        A_sb[i][:],
                        start=(i == 0),
                        stop=(i == NI - 1),
                    )
                m1 = io.tile([P, CHUNK], bc_dt, name=f"M1{jj}")
                nc.scalar.copy(m1[:], psm[:])
                M1_sb.append(m1)

            # ---- stage C: Y = M1 @ Wp  (natural layout) ----
            for tt in range(NT):
                psy = ps_pool.tile([P, C], F32, name=f"psY{tt}", tag="big")
                st = slice(tt * P, (tt + 1) * P)
                for jj in range(NI):
                    nc.tensor.matmul(
                        psy[:],
                        M1_sb[jj][:, st],
                        wpb[jj][:],
                        start=(jj == 0),
                        stop=(jj == NI - 1),
                    )
                ysb = io.tile([P, C], F32, name=f"y{tt}")
                nc.scalar.copy(ysb[:], psy[:])
                nc.sync.dma_start(y_ap[st, :], ysb[:])

    nc.compile()
    return nc


def _get_nc():
    key = MODE[0]
    if key not in _STATE:
        _STATE[key] = _build_nc(key)
    return _STATE[key]


def _prepare_in_maps(x, w_attn, w_proj):
    x = np.asarray(x, dtype=np.float32)
    w_attn = np.asarray(w_attn, dtype=np.float32)
    w_proj = np.ascontiguousarray(np.asarray(w_proj, dtype=np.float32))
    wv = np.ascontiguousarray(w_attn[:, 2 * C : 3 * C])

    in_maps = []
    for core in range(N_CORES):
        b, tc = divmod(core, T // CHUNK)
        goff = tc * CHUNK
        chunk = np.ascontiguousarray(x[b, goff : goff + CHUNK, :])
        # halo: column-sum of all earlier rows in this batch element
        p = x[b, :goff, :].sum(axis=0, dtype=np.float32) if goff else np.zeros(
            C, np.float32
        )
        pc = np.ascontiguousarray(p.reshape(NI, P).T)  # pc[r, i] = p[i*P + r]
        scale = (1.0 / (goff + np.arange(1, CHUNK + 1))).astype(np.float32)
        sc = np.ascontiguousarray(np.broadcast_to(scale, (P, CHUNK)))
        us = np.zeros((P, CHUNK), np.float32)
        tri = np.triu(np.ones((P, P), np.float32))  # s <= t
        for j in range(NT):
            us[:, j * P : (j + 1) * P] = tri * scale[j * P : (j + 1) * P][None, :]
        # colsum recovery: psA[:, j*P+P-1] * (goff + j*P + P) == tile colsum
        rcv = (goff + (np.arange(NT) + 1.0) * P).astype(np.float32)
        rc = np.ascontiguousarray(np.broadcast_to(rcv, (P, NT)))
        in_maps.append(
            {
                "x": chunk, "wv": wv, "wp": w_proj,
                "us": us, "sc": sc, "pc": pc, "rc": rc,
            }
        )
    return in_maps


def kernel(x, w_attn, w_proj):
    nc = _get_nc()
    in_maps = _prepare_in_maps(x, w_attn, w_proj)
    res = bass_utils.run_bass_kernel_spmd(
        nc, in_maps, core_ids=list(range(N_CORES)), trace=TRACE[0]
    )
    LAST_RESULT[0] = res
    y = np.empty((B, T, C), np.float32)
    for core in range(N_CORES):
        b, tc = divmod(core, T // CHUNK)
        y[b, tc * CHUNK : (tc + 1) * CHUNK, :] = res.results[core]["y"]
    return y


# revision 5
# speedup vs baseline: 1.6177x; 1.6177x over previous
"""Trainium2 Bass kernel for nn_CausalSelfAttention_74268574482879.

The reference module's attention scores are overwritten by the causal mask
(q/k are discarded), so softmax weights are uniform over positions <= t:
    y = cummean_T(x) @ W_v @ W_p,   W_v = w_attn[:, 1024:1536]

Distribution: the 4096 rows of (B*T) are split into 8 chunks of 512 rows,
one per NeuronCore.  The only cross-chunk dependency is the column-sum of
all preceding rows in the same batch element; the host passes that tiny
(512,) halo vector per core while slicing the shards.

Per-core dataflow (matmuls keep operands in natural layout — the PE's
implicit transpose of the stationary operand does all layout work):
  stage A: lhsT=x_tile, rhs=U_scaled  ->  psA = scale*(local cumsum)^T (PSUM)
           tile colsums are recovered from psA's last column (one fused
           tensor_scalar each), then a DVE/GpSimd carry adds P_j[c]*scale[t]
  stage B: lhsT=Wv,     rhs=A^T       ->  M1^T = (A @ Wv)^T
  stage C: lhsT=M1^T,   rhs=Wp        ->  Y = M1 @ Wp   (natural, DMA out)
"""

import numpy as np

import concourse.bass as bass
import concourse.bacc as bacc
import concourse.mybir as mybir
import concourse.tile as tile
from concourse import bass_utils

N_CORES = 8
B, T, C = 2, 2048, 512
CHUNK = 512               # rows of flattened (B*T) per core
P = 128
NT = CHUNK // P           # 4 row-tiles per chunk
NI = C // P               # 4 col-tiles of the 512 feature dim
F32 = mybir.dt.float32
F32R = mybir.dt.float32r
BF16 = mybir.dt.bfloat16

MODE = ["f32r"]           # "f32" | "f32r" | "bf16" (stage B/C dtype)
TRACE = [False]
LAST_RESULT = [None]
_STATE = {}


def _build_nc(mode):
    nc = bacc.Bacc(
        "TRN2", target_bir_lowering=False, debug=False, num_devices=N_CORES
    )
    # dtype plan per mode: a_dt feeds stage-A matmuls, bc_dt feeds B/C.
    # float32r keeps fp32 bits but runs the PE in single-pass reduced mode;
    # the verifier wants every producer of a matmul operand to declare it.
    if mode == "f32":
        a_dt, bc_dt = F32, F32
    elif mode == "f32r":
        a_dt, bc_dt = F32R, F32R
    else:  # bf16 B/C, f32r stage A
        a_dt, bc_dt = F32R, BF16
    bc_bf16 = bc_dt == BF16
    wdma_dt = F32 if bc_bf16 else bc_dt

    x_d = nc.dram_tensor("x", (CHUNK, C), a_dt, kind="ExternalInput")
    wv_d = nc.dram_tensor("wv", (C, C), wdma_dt, kind="ExternalInput")
    wp_d = nc.dram_tensor("wp", (C, C), wdma_dt, kind="ExternalInput")
    us_d = nc.dram_tensor("us", (P, CHUNK), a_dt, kind="ExternalInput")
    sc_d = nc.dram_tensor("sc", (P, CHUNK), F32, kind="ExternalInput")
    pc_d = nc.dram_tensor("pc", (P, NI), F32, kind="ExternalInput")
    rc_d = nc.dram_tensor("rc", (P, NT), F32, kind="ExternalInput")
    y_d = nc.dram_tensor("y", (CHUNK, C), F32, kind="ExternalOutput")

    x_ap, wv_ap, wp_ap = x_d.ap(), wv_d.ap(), wp_d.ap()
    us_ap, sc_ap, pc_ap, rc_ap, y_ap = (
        us_d.ap(), sc_d.ap(), pc_d.ap(), rc_d.ap(), y_d.ap()
    )

    with tile.TileContext(nc) as tc:
        with (
            tc.tile_pool(name="io", bufs=1) as io,
            tc.tile_pool(name="tmp", bufs=4) as tmp_pool,
            tc.tile_pool(name="psbig", bufs=8, space="PSUM") as ps_pool,
        ):
            # ---- inputs to SBUF (order = DMA priority) ----
            x_sb = [io.tile([P, C], a_dt, name=f"x{k}") for k in range(NT)]
            nc.sync.dma_start(x_sb[0][:], x_ap[0:P, :])
            us_sb = io.tile([P, CHUNK], a_dt, name="us_sb")
            nc.sync.dma_start(us_sb[:], us_ap[:, :])
            for k in range(1, NT):
                nc.sync.dma_start(x_sb[k][:], x_ap[k * P : (k + 1) * P, :])
            pc_sb = io.tile([P, NI], F32, name="pc_sb")
            nc.sync.dma_start(pc_sb[:], pc_ap[:, :])
            rc_sb = io.tile([P, NT], F32, name="rc_sb")
            nc.sync.dma_start(rc_sb[:], rc_ap[:, :])
            sc_sb = io.tile([P, CHUNK], F32, name="sc_sb")
            nc.sync.dma_start(sc_sb[:], sc_ap[:, :])
            wv_sb, wp_sb = [], []
            for i in range(NI):
                wvi = io.tile([P, C], wdma_dt, name=f"wv{i}")
                nc.sync.dma_start(wvi[:], wv_ap[i * P : (i + 1) * P, :])
                wv_sb.append(wvi)
            for j in range(NI):
                wpj = io.tile([P, C], wdma_dt, name=f"wp{j}")
                nc.sync.dma_start(wpj[:], wp_ap[j * P : (j + 1) * P, :])
                wp_sb.append(wpj)

            if bc_bf16:
                wvb, wpb = [], []
                for i in range(NI):
                    t = io.tile([P, C], BF16, name=f"wvb{i}")
                    nc.scalar.copy(t[:], wv_sb[i][:])
                    wvb.append(t)
                for j in range(NI):
                    t = io.tile([P, C], BF16, name=f"wpb{j}")
                    nc.scalar.copy(t[:], wp_sb[j][:])
                    wpb.append(t)
            else:
                wvb, wpb = wv_sb, wp_sb

            # ---- stage A: scaled local cumsum (PE), j-outer so the first
            # round only needs x_sb[0] ----
            psA = [
                ps_pool.tile([P, CHUNK], F32, name=f"psA{i}", tag="big")
                for i in range(NI)
            ]
            for j in range(NT):
                tj = slice(j * P, (j + 1) * P)
                for i in range(NI):
                    ci = slice(i * P, (i + 1) * P)
                    nc.tensor.matmul(
                        psA[i][:, tj],
                        x_sb[j][:, ci],
                        us_sb[:, tj],
                        start=True,
                        stop=True,
                    )

            # ---- running prefixes Pc[:, i*NT+j] = p + sum_{k<j} colsum_k,
            # recovering colsum_k from psA's last column of tile k ----
            Pc_sb = io.tile([P, NT * NI], F32, name="Pc_sb")
            for i in range(NI):
                nc.vector.tensor_copy(
                    Pc_sb[:, i * NT : i * NT + 1], pc_sb[:, i : i + 1]
                )
                for j in range(1, NT):
                    lcol = (j - 1) * P + P - 1
                    nc.vector.tensor_scalar(
                        Pc_sb[:, i * NT + j : i * NT + j + 1],
                        psA[i][:, lcol : lcol + 1],
                        rc_sb[:, j - 1 : j],
                        Pc_sb[:, i * NT + j - 1 : i * NT + j],
                        mybir.AluOpType.mult,
                        mybir.AluOpType.add,
                    )

            # ---- carry: A^T = psA + Pc * scale  (GpSimd mul + DVE add) ----
            A_sb = []
            for i in range(NI):
                ai = io.tile([P, CHUNK], bc_dt, name=f"A{i}")
                for j in range(NT):
                    tj = slice(j * P, (j + 1) * P)
                    col = i * NT + j
                    tmp = tmp_pool.tile([P, P], F32, name="carry", tag="carry")
                    nc.vector.tensor_scalar_mul(
                        tmp[:], sc_sb[:, tj], Pc_sb[:, col : col + 1]
                    )
                    nc.vector.tensor_add(ai[:, tj], psA[i][:, tj], tmp[:])
                A_sb.append(ai)

            # ---- stage B: M1^T = (A @ Wv)^T ----
            M1_sb = []
            for jj in range(NI):
                psm = ps_pool.tile([P, CHUNK], F32, name=f"psM{jj}", tag="big")
                cj = slice(jj * P, (jj + 1) * P)
                for i in range(NI):
                    nc.tensor.matmul(
                        psm[:],
                        wvb[i][:, cj],
                        A_sb[i][:],
                        start=(i == 0),
                        stop=(i == NI - 1),
                    )
                m1 = io.tile([P, CHUNK], bc_dt, name=f"M1{jj}")
                nc.vector.tensor_copy(m1[:], psm[:])
                M1_sb.append(m1)

            # ---- stage C: Y = M1 @ Wp  (natural layout) ----
            for tt in range(NT):
                psy = ps_pool.tile([P, C], F32, name=f"psY{tt}", tag="big")
                st = slice(tt * P, (tt + 1) * P)
                for jj in range(NI):
                    nc.tensor.matmul(
                        psy[:],
                        M1_sb[jj][:, st],
                        wpb[jj][:],
                        start=(jj == 0),
                        stop=(jj == NI - 1),
                    )
                ysb = io.tile([P, C], F32, name=f"y{tt}")
                nc.scalar.copy(ysb[:], psy[:])
                nc.sync.dma_start(y_ap[st, :], ysb[:])

    nc.compile()
    return nc


def _get_nc():
    key = MODE[0]
    if key not in _STATE:
        _STATE[key] = _build_nc(key)
    return _STATE[key]


def _prepare_in_maps(x, w_attn, w_proj):
    x = np.asarray(x, dtype=np.float32)
    w_attn = np.asarray(w_attn, dtype=np.float32)
    w_proj = np.ascontiguousarray(np.asarray(w_proj, dtype=np.float32))
    wv = np.ascontiguousarray(w_attn[:, 2 * C : 3 * C])

    in_maps = []
    for core in range(N_CORES):
        b, tc = divmod(core, T // CHUNK)
        goff = tc * CHUNK
        chunk = np.ascontiguousarray(x[b, goff : goff + CHUNK, :])
        # halo: column-sum of all earlier rows in this batch element
        p = x[b, :goff, :].sum(axis=0, dtype=np.float32) if goff else np.zeros(
            C, np.float32
        )
        pc = np.ascontiguousarray(p.reshape(NI, P).T)  # pc[r, i] = p[i*P + r]
        scale = (1.0 / (goff + np.arange(1, CHUNK + 1))).astype(np.float32)
        sc = np.ascontiguousarray(np.broadcast_to(scale, (P, CHUNK)))
        us = np.zeros((P, CHUNK), np.float32)
        tri = np.triu(np.ones((P, P), np.float32))  # s <= t
        for j in range(NT):
            us[:, j * P : (j + 1) * P] = tri * scale[j * P : (j + 1) * P][None, :]
        # colsum recovery: psA[:, j*P+P-1] * (goff + j*P + P) == tile colsum
        rcv = (goff + (np.arange(NT) + 1.0) * P).astype(np.float32)
        rc = np.ascontiguousarray(np.broadcast_to(rcv, (P, NT)))
        in_maps.append(
            {
                "x": chunk, "wv": wv, "wp": w_proj,
                "us": us, "sc": sc, "pc": pc, "rc": rc,
            }
        )
    return in_maps


def kernel(x, w_attn, w_proj):
    nc = _get_nc()
    in_maps = _prepare_in_maps(x, w_attn, w_proj)
    res = bass_utils.run_bass_kernel_spmd(
        nc, in_maps, core_ids=list(range(N_CORES)), trace=TRACE[0]
    )
    LAST_RESULT[0] = res
    y = np.empty((B, T, C), np.float32)
    for core in range(N_CORES):
        b, tc = divmod(core, T // CHUNK)
        y[b, tc * CHUNK : (tc + 1) * CHUNK, :] = res.results[core]["y"]
    return y


# revision 7
# speedup vs baseline: 1.6246x; 1.0043x over previous
"""Trainium2 Bass kernel for nn_CausalSelfAttention_74268574482879.

The reference module's attention scores are overwritten by the causal mask
(q/k are discarded), so softmax weights are uniform over positions <= t:
    y = cummean_T(x) @ W_v @ W_p,   W_v = w_attn[:, 1024:1536]

Distribution: the 4096 rows of (B*T) are split into 8 chunks of 512 rows,
one per NeuronCore.  The only cross-chunk dependency is the column-sum of
all preceding rows in the same batch element; the host passes that tiny
(512,) halo vector per core while slicing the shards.

Per-core dataflow (matmuls keep operands in natural layout — the PE's
implicit transpose of the stationary operand does all layout work):
  stage A: lhsT=x_tile, rhs=U_scaled  ->  psA = scale*(local cumsum)^T (PSUM)
           tile colsums are recovered from psA's last column (one fused
           tensor_scalar each), then a DVE/GpSimd carry adds P_j[c]*scale[t]
  stage B: lhsT=Wv,     rhs=A^T       ->  M1^T = (A @ Wv)^T
  stage C: lhsT=M1^T,   rhs=Wp        ->  Y = M1 @ Wp   (natural, DMA out)
"""

import numpy as np

import concourse.bass as bass
import concourse.bacc as bacc
import concourse.mybir as mybir
import concourse.tile as tile
from concourse import bass_utils

N_CORES = 8
B, T, C = 2, 2048, 512
CHUNK = 512               # rows of flattened (B*T) per core
P = 128
NT = CHUNK // P           # 4 row-tiles per chunk
NI = C // P               # 4 col-tiles of the 512 feature dim
F32 = mybir.dt.float32
F32R = mybir.dt.float32r
BF16 = mybir.dt.bfloat16

MODE = ["f32r"]           # "f32" | "f32r" | "bf16" (stage B/C dtype)
TRACE = [False]
LAST_RESULT = [None]
_STATE = {}


def _build_nc(mode):
    nc = bacc.Bacc(
        "TRN2", target_bir_lowering=False, debug=False, num_devices=N_CORES
    )
    # dtype plan per mode: a_dt feeds stage-A matmuls, bc_dt feeds B/C.
    # float32r keeps fp32 bits but runs the PE in single-pass reduced mode;
    # the verifier wants every producer of a matmul operand to declare it.
    if mode == "f32":
        a_dt, bc_dt = F32, F32
    elif mode == "f32r":
        a_dt, bc_dt = F32R, F32R
    else:  # bf16 B/C, f32r stage A
        a_dt, bc_dt = F32R, BF16
    bc_bf16 = bc_dt == BF16
    wdma_dt = F32 if bc_bf16 else bc_dt

    x_d = nc.dram_tensor("x", (CHUNK, C), a_dt, kind="ExternalInput")
    wv_d = nc.dram_tensor("wv", (C, C), wdma_dt, kind="ExternalInput")
    wp_d = nc.dram_tensor("wp", (C, C), wdma_dt, kind="ExternalInput")
    us_d = nc.dram_tensor("us", (P, CHUNK), a_dt, kind="ExternalInput")
    sc_d = nc.dram_tensor("sc", (P, CHUNK), F32, kind="ExternalInput")
    pc_d = nc.dram_tensor("pc", (P, NI), F32, kind="ExternalInput")
    rc_d = nc.dram_tensor("rc", (P, NT), F32, kind="ExternalInput")
    y_d = nc.dram_tensor("y", (CHUNK, C), F32, kind="ExternalOutput")

    x_ap, wv_ap, wp_ap = x_d.ap(), wv_d.ap(), wp_d.ap()
    us_ap, sc_ap, pc_ap, rc_ap, y_ap = (
        us_d.ap(), sc_d.ap(), pc_d.ap(), rc_d.ap(), y_d.ap()
    )

    with tile.TileContext(nc) as tc:
        with (
            tc.tile_pool(name="io", bufs=1) as io,
            tc.tile_pool(name="tmp", bufs=4) as tmp_pool,
            tc.tile_pool(name="psbig", bufs=8, space="PSUM") as ps_pool,
        ):
            # ---- inputs to SBUF (order = DMA priority) ----
            # x packed as one (P, NT, C) tile: block k holds rows k*P..k*P+P
            x_pack = io.tile([P, NT, C], a_dt, name="x_pack")
            nc.sync.dma_start(x_pack[:, 0, :], x_ap[0:P, :])
            us_sb = io.tile([P, CHUNK], a_dt, name="us_sb")
            nc.sync.dma_start(us_sb[:], us_ap[:, :])
            nc.sync.dma_start(
                x_pack[:, 1:, :],
                x_ap.rearrange("(k p) c -> p k c", p=P)[:, 1:, :],
            )
            x_sb = [x_pack[:, k, :] for k in range(NT)]
            pc_sb = io.tile([P, NI], F32, name="pc_sb")
            nc.sync.dma_start(pc_sb[:], pc_ap[:, :])
            rc_sb = io.tile([P, NT], F32, name="rc_sb")
            nc.sync.dma_start(rc_sb[:], rc_ap[:, :])
            sc_sb = io.tile([P, CHUNK], F32, name="sc_sb")
            nc.sync.dma_start(sc_sb[:], sc_ap[:, :])
            wv_pack = io.tile([P, NI, C], wdma_dt, name="wv_pack")
            nc.sync.dma_start(wv_pack[:], wv_ap.rearrange("(k p) c -> p k c", p=P))
            wp_pack = io.tile([P, NI, C], wdma_dt, name="wp_pack")
            nc.sync.dma_start(wp_pack[:], wp_ap.rearrange("(k p) c -> p k c", p=P))
            wv_sb = [wv_pack[:, i, :] for i in range(NI)]
            wp_sb = [wp_pack[:, j, :] for j in range(NI)]

            if bc_bf16:
                wvb, wpb = [], []
                for i in range(NI):
                    t = io.tile([P, C], BF16, name=f"wvb{i}")
                    nc.scalar.copy(t[:], wv_sb[i][:])
                    wvb.append(t)
                for j in range(NI):
                    t = io.tile([P, C], BF16, name=f"wpb{j}")
                    nc.scalar.copy(t[:], wp_sb[j][:])
                    wpb.append(t)
            else:
                wvb, wpb = wv_sb, wp_sb

            # ---- stage A: scaled local cumsum (PE), j-outer so the first
            # round only needs x_sb[0] ----
            psA = [
                ps_pool.tile([P, CHUNK], F32, name=f"psA{i}", tag="big")
                for i in range(NI)
            ]
            for i in range(NI):
                ci = slice(i * P, (i + 1) * P)
                for j in range(NT):
                    tj = slice(j * P, (j + 1) * P)
                    nc.tensor.matmul(
                        psA[i][:, tj],
                        x_sb[j][:, ci],
                        us_sb[:, tj],
                        start=True,
                        stop=True,
                    )

            # ---- running prefixes Pc[:, i*NT+j] = p + sum_{k<j} colsum_k,
            # recovering colsum_k from psA's last column of tile k ----
            Pc_sb = io.tile([P, NT * NI], F32, name="Pc_sb")
            for i in range(NI):
                nc.vector.tensor_copy(
                    Pc_sb[:, i * NT : i * NT + 1], pc_sb[:, i : i + 1]
                )
                for j in range(1, NT):
                    lcol = (j - 1) * P + P - 1
                    nc.vector.tensor_scalar(
                        Pc_sb[:, i * NT + j : i * NT + j + 1],
                        psA[i][:, lcol : lcol + 1],
                        rc_sb[:, j - 1 : j],
                        Pc_sb[:, i * NT + j - 1 : i * NT + j],
                        mybir.AluOpType.mult,
                        mybir.AluOpType.add,
                    )

            # ---- carry: A^T = psA + Pc * scale  (GpSimd mul + DVE add) ----
            A_sb = []
            for i in range(NI):
                ai = io.tile([P, CHUNK], bc_dt, name=f"A{i}")
                for j in range(NT):
                    tj = slice(j * P, (j + 1) * P)
                    col = i * NT + j
                    tmp = tmp_pool.tile([P, P], F32, name="carry", tag="carry")
                    nc.vector.tensor_scalar_mul(
                        tmp[:], sc_sb[:, tj], Pc_sb[:, col : col + 1]
                    )
                    nc.vector.tensor_add(ai[:, tj], psA[i][:, tj], tmp[:])
                A_sb.append(ai)

            # ---- stage B: M1^T = (A @ Wv)^T ----
            M1_sb = []
            for jj in range(NI):
                psm = ps_pool.tile([P, CHUNK], F32, name=f"psM{jj}", tag="big")
                cj = slice(jj * P, (jj + 1) * P)
                for i in range(NI):
                    nc.tensor.matmul(
                        psm[:],
                        wvb[i][:, cj],
                        A_sb[i][:],
                        start=(i == 0),
                        stop=(i == NI - 1),
                    )
                m1 = io.tile([P, CHUNK], bc_dt, name=f"M1{jj}")
                nc.vector.tensor_copy(m1[:], psm[:])
                M1_sb.append(m1)

            # ---- stage C: Y = M1 @ Wp  (natural layout) ----
            for tt in range(NT):
                psy = ps_pool.tile([P, C], F32, name=f"psY{tt}", tag="big")
                st = slice(tt * P, (tt + 1) * P)
                for jj in range(NI):
                    nc.tensor.matmul(
                        psy[:],
                        M1_sb[jj][:, st],
                        wpb[jj][:],
                        start=(jj == 0),
                        stop=(jj == NI - 1),
                    )
                ysb = io.tile([P, C], F32, name=f"y{tt}")
                nc.scalar.copy(ysb[:], psy[:])
                nc.sync.dma_start(y_ap[st, :], ysb[:])

    nc.compile()
    return nc


def _get_nc():
    key = MODE[0]
    if key not in _STATE:
        _STATE[key] = _build_nc(key)
    return _STATE[key]


def _prepare_in_maps(x, w_attn, w_proj):
    x = np.asarray(x, dtype=np.float32)
    w_attn = np.asarray(w_attn, dtype=np.float32)
    w_proj = np.ascontiguousarray(np.asarray(w_proj, dtype=np.float32))
    wv = np.ascontiguousarray(w_attn[:, 2 * C : 3 * C])

    in_maps = []
    for core in range(N_CORES):
        b, tc = divmod(core, T // CHUNK)
        goff = tc * CHUNK
        chunk = np.ascontiguousarray(x[b, goff : goff + CHUNK, :])
        # halo: column-sum of all earlier rows in this batch element
        p = x[b, :goff, :].sum(axis=0, dtype=np.float32) if goff else np.zeros(
            C, np.float32
        )
        pc = np.ascontiguousarray(p.reshape(NI, P).T)  # pc[r, i] = p[i*P + r]
        scale = (1.0 / (goff + np.arange(1, CHUNK + 1))).astype(np.float32)
        sc = np.ascontiguousarray(np.broadcast_to(scale, (P, CHUNK)))
        us = np.zeros((P, CHUNK), np.float32)
        tri = np.triu(np.ones((P, P), np.float32))  # s <= t
        for j in range(NT):
            us[:, j * P : (j + 1) * P] = tri * scale[j * P : (j + 1) * P][None, :]
        # colsum recovery: psA[:, j*P+P-1] * (goff + j*P + P) == tile colsum
        rcv = (goff + (np.arange(NT) + 1.0) * P).astype(np.float32)
        rc = np.ascontiguousarray(np.broadcast_to(rcv, (P, NT)))
        in_maps.append(
            {
                "x": chunk, "wv": wv, "wp": w_proj,
                "us": us, "sc": sc, "pc": pc, "rc": rc,
            }
        )
    return in_maps


def kernel(x, w_attn, w_proj):
    nc = _get_nc()
    in_maps = _prepare_in_maps(x, w_attn, w_proj)
    res = bass_utils.run_bass_kernel_spmd(
        nc, in_maps, core_ids=list(range(N_CORES)), trace=TRACE[0]
    )
    LAST_RESULT[0] = res
    y = np.empty((B, T, C), np.float32)
    for core in range(N_CORES):
        b, tc = divmod(core, T // CHUNK)
        y[b, tc * CHUNK : (tc + 1) * CHUNK, :] = res.results[core]["y"]
    return y
